# revision 8
# baseline (speedup 1.0000x reference)
"""Trainium2 Bass kernel for nn_EquivariantDecoder (EGNN, B=2, N=192, 4 layers).

Sharding: 8 cores = 2 graphs x 4 dst-chunks of 48. Each core computes all
(src, dst) pairs for its 48 dsts; h/x are re-assembled between layers with an
AllGather over the 4-core group of each graph.

Algebraic restructuring (vs the dense reference):
- edge-MLP second layer fused into the pair-MLP first layers:
    W_ne = nw1_e @ ew2 (host-side), const_n = nb1 + nw1_e @ eb2
- masked pairs: a dedicated contraction channel carries -BIG into the
  pre-activation, so silu(pre) ~= 0 exactly for masked pairs (no per-pair
  mask multiply anywhere downstream)
- h[dst]-projection bias b[h,d] folded into the SAME first matmul via 48
  extra contraction channels (lhsT rows 64..111 = b^T, rhs rows = onehot(d))
- aggregation swapped before the second matmul:
    h_agg = nw2 @ (sum_s silu(pre_n)) + nb2 * cnt(d)
- coordinate update factored:  x_agg[:,d] = X @ w'[:,d] - x[:,d] * S(d)
  with w'[s,d] = cw2 @ silu(pre_c) (auto-masked by -BIG), S = sum_s w'.

Pair order is s-major: p = s*48 + d_local. PSUM blocks hold 10 srcs x 48 dsts
(480 cols) per 512-col bank.

Slab layout [113, 9216] (shared rhs for the first matmuls, K=113):
  rows 0..63   e1s = silu(D*ew1 + eb1)  (rewritten per layer)
  rows 64..111 onehot(d_local) pattern  (static)
  row  112     1 - mask                 (static; lhsT row = -BIG)
"""

import numpy as np

# ---- problem constants (hardcoded; kernel.py must be self-contained) ----
B = 2
N = 192
ND = 128          # node dim
NH = 256          # hidden
NE = 64           # edge dim
L = 4
CUT = 2.0
MY = 48           # dsts per core
BIG = 1e30
N_CORES = 8
PAIRS = N * MY    # 9216

# s-blocks: 10 srcs x 48 dsts = 480 cols per PSUM bank
_BLOCKS = [(s0, min(10, N - s0)) for s0 in range(0, N, 10)]  # 19x10 + 1x2
# supertiles: up to 3 blocks each
_SUPER = [_BLOCKS[i:i + 3] for i in range(0, len(_BLOCKS), 3)]


def _build(l_eff=L, debug_h=False, noskip=False):
    import concourse.bass as bass
    import concourse.bacc as bacc
    import concourse.tile as tile
    import concourse.mybir as mybir

    F32 = mybir.dt.float32
    AF = mybir.ActivationFunctionType
    OP = mybir.AluOpType

    nc = bacc.Bacc(
        "TRN2",
        target_bir_lowering=False,
        debug=False,
        enable_asserts=False,
        num_devices=N_CORES,
    )

    def din(name, shape):
        return nc.dram_tensor(name, shape, F32, kind="ExternalInput")

    d_hT = din("hT", [ND, N])
    d_hT_my = din("hT_my", [ND, MY])
    d_eye = din("eye_my", [N, MY])
    d_onehot = din("onehot_slab", [MY, PAIRS])
    d_ident = din("ident128", [ND, ND])
    d_initwT = din("init_wT", [ND, 3])
    d_initb = din("init_b_row", [1, 3])
    d_ew1 = din("ew1_pack", [1, L * NE])
    d_eb1 = din("eb1_pack", [NE, L])
    d_wnext = din("wnext_pack", [65, L * NH])
    d_wcext = din("wcext_pack", [65, L * NH])
    d_nw1sT = din("nw1sT_pack", [ND, L * NH])
    d_cw1sT = din("cw1sT_pack", [ND, L * NH])
    d_nw1dT = din("nw1dT_pack", [ND, L * NH])
    d_cw1dT = din("cw1dT_pack", [ND, L * NH])
    d_constn = din("constn_pack", [1, L * NH])
    d_constc = din("constc_pack", [1, L * NH])
    d_nw2T = din("nw2T_pack", [ND, L * 2 * ND])
    d_nb2 = din("nb2_pack", [1, L * ND])
    d_cw2 = din("cw2_pack", [ND, L * 2])

    d_out = nc.dram_tensor("x_out", [3, MY], F32, kind="ExternalOutput")
    d_hout = None
    if debug_h:
        d_hout = nc.dram_tensor("h_out", [ND, MY], F32, kind="ExternalOutput")

    with tile.TileContext(nc) as tc:
        with (
            tc.tile_pool(name="stat", bufs=1) as stat,
            tc.tile_pool(name="rot", bufs=3) as rot,
            tc.tile_pool(name="psA", bufs=2, space="PSUM") as psA,
            tc.tile_pool(name="psB", bufs=2, space="PSUM") as psB,
            tc.tile_pool(name="dram", bufs=1, space="DRAM") as dram,
        ):
            # ---------------- static SBUF loads ----------------
            def sload(d, shape, name):
                t = stat.tile(shape, F32, name=name, tag=name)
                nc.sync.dma_start(t, d[:])
                return t

            hT = sload(d_hT, [ND, N], "hT")
            hT_myA = sload(d_hT_my, [ND, MY], "hT_myA")
            hT_myB = stat.tile([ND, MY], F32, name="hT_myB", tag="hT_myB")
            eye0 = stat.tile([128, MY], F32, name="eye0", tag="eye0")
            eye1 = stat.tile([64, MY], F32, name="eye1", tag="eye1")
            nc.sync.dma_start(eye0, d_eye[0:128, :])
            nc.sync.dma_start(eye1, d_eye[128:192, :])
            ident = sload(d_ident, [ND, ND], "ident")
            initwT = sload(d_initwT, [ND, 3], "initwT")
            initb = sload(d_initb, [1, 3], "initb")
            ew1 = sload(d_ew1, [1, L * NE], "ew1")
            eb1 = sload(d_eb1, [NE, L], "eb1")
            nw1sT = sload(d_nw1sT, [ND, L * NH], "nw1sT")
            cw1sT = sload(d_cw1sT, [ND, L * NH], "cw1sT")
            nw1dT = sload(d_nw1dT, [ND, L * NH], "nw1dT")
            cw1dT = sload(d_cw1dT, [ND, L * NH], "cw1dT")
            constn = sload(d_constn, [1, L * NH], "constn")
            constc = sload(d_constc, [1, L * NH], "constc")
            nw2T = sload(d_nw2T, [ND, L * 2 * ND], "nw2T")
            nb2 = sload(d_nb2, [1, L * ND], "nb2")
            cw2 = sload(d_cw2, [ND, L * 2], "cw2")

            slab = stat.tile([113, PAIRS], F32, name="slab", tag="slab")
            nc.sync.dma_start(slab[64:112, :], d_onehot[:])

            # lhsT tiles [113, 128] per (layer, path, chunk)
            lhsT_n = {}
            lhsT_c = {}
            for l in range(l_eff):
                for ch in range(2):
                    sl = slice(l * NH + ch * ND, l * NH + (ch + 1) * ND)
                    tn = stat.tile([113, ND], F32, name=f"lhsTn_{l}_{ch}",
                                   tag=f"lhsTn_{l}_{ch}")
                    nc.sync.dma_start(tn[0:64, :], d_wnext[0:64, sl])
                    nc.sync.dma_start(tn[112:113, :], d_wnext[64:65, sl])
                    lhsT_n[(l, ch)] = tn
                    tcn = stat.tile([113, ND], F32, name=f"lhsTc_{l}_{ch}",
                                    tag=f"lhsTc_{l}_{ch}")
                    nc.sync.dma_start(tcn[0:64, :], d_wcext[0:64, sl])
                    nc.sync.dma_start(tcn[112:113, :], d_wcext[64:65, sl])
                    lhsT_c[(l, ch)] = tcn

            ones_row = stat.tile([1, N], F32, name="ones_row", tag="ones_row")
            nc.vector.memset(ones_row, 1.0)
            ones_col = stat.tile([128, 3], F32, name="ones_col", tag="ones_col")
            nc.vector.memset(ones_col, 1.0)

            # ---------------- init: coords, D, mask ----------------
            xf_ps = psB.tile([3, N], F32, tag="small", name="xf_ps")
            nc.tensor.matmul(xf_ps, initwT, hT, start=True, stop=False)
            nc.tensor.matmul(xf_ps, initb, ones_row, start=False, stop=True)
            Xf = stat.tile([3, N], F32, name="Xf", tag="Xf")
            nc.vector.tensor_copy(Xf, xf_ps)

            xfm_ps = psB.tile([3, MY], F32, tag="small", name="xfm_ps")
            nc.tensor.matmul(xfm_ps, initwT, hT_myA, start=True, stop=False)
            nc.tensor.matmul(xfm_ps, initb, ones_row[0:1, 0:MY],
                             start=False, stop=True)
            Xf_myA = stat.tile([3, MY], F32, name="Xf_myA", tag="Xf_myA")
            Xf_myB = stat.tile([3, MY], F32, name="Xf_myB", tag="Xf_myB")
            nc.vector.tensor_copy(Xf_myA, xfm_ps)

            # n2 = sum_i x_i^2 as [1, N] via K=3 matmul with ones
            sqX = stat.tile([3, N], F32, name="sqX", tag="sqX")
            nc.vector.tensor_mul(sqX, Xf, Xf)
            n2_ps = psB.tile([1, N], F32, tag="small", name="n2_ps")
            nc.tensor.matmul(n2_ps, ones_col[0:3, 0:1], sqX, start=True, stop=True)
            n2 = stat.tile([1, N], F32, name="n2", tag="n2")
            nc.vector.tensor_copy(n2, n2_ps)
            sqXm = stat.tile([3, MY], F32, name="sqXm", tag="sqXm")
            nc.vector.tensor_mul(sqXm, Xf_myA, Xf_myA)
            n2m_ps = psB.tile([1, MY], F32, tag="small", name="n2m_ps")
            nc.tensor.matmul(n2m_ps, ones_col[0:3, 0:1], sqXm, start=True, stop=True)
            n2m = stat.tile([1, MY], F32, name="n2m", tag="n2m")
            nc.vector.tensor_copy(n2m, n2m_ps)

            # P [5, N]: rows 0-2 = -2x, row 3 = ones (DMA), row 4 = n2 (DMA)
            P = stat.tile([5, N], F32, name="P", tag="P")
            nc.vector.tensor_scalar_mul(P[0:3, :], Xf, -2.0)
            nc.sync.dma_start(P[3:4, :], ones_row)
            nc.sync.dma_start(P[4:5, :], n2)
            # Q [5, MY]: rows 0-2 = x_my (DMA), row 3 = n2m (DMA), row 4 = ones
            Q = stat.tile([5, MY], F32, name="Q", tag="Q")
            nc.vector.tensor_copy(Q[0:3, :], Xf_myA)
            nc.sync.dma_start(Q[3:4, :], n2m)
            nc.sync.dma_start(Q[4:5, :], ones_row[0:1, 0:MY])

            D_sd = [stat.tile([128, MY], F32, name="D0", tag="D0"),
                    stat.tile([64, MY], F32, name="D1", tag="D1")]
            om = [stat.tile([128, MY], F32, name="om0", tag="om0"),
                  stat.tile([64, MY], F32, name="om1", tag="om1")]
            mask_s = [stat.tile([128, MY], F32, name="ms0", tag="ms0"),
                      stat.tile([64, MY], F32, name="ms1", tag="ms1")]
            for ci, (p0, pn) in enumerate([(0, 128), (128, 64)]):
                sq_ps = psB.tile([pn, MY], F32, tag="small", name=f"sqps{ci}")
                nc.tensor.matmul(sq_ps, P[:, p0:p0 + pn], Q, start=True, stop=True)
                sq = rot.tile([pn, MY], F32, name=f"sq{ci}", tag=f"sq{ci}")
                nc.vector.tensor_scalar_max(sq, sq_ps, 0.0)
                y = rot.tile([pn, MY], F32, name=f"y{ci}", tag=f"y{ci}")
                nc.scalar.activation(y, sq, AF.Sqrt)
                r = rot.tile([pn, MY], F32, name=f"r{ci}", tag=f"r{ci}")
                t = rot.tile([pn, MY], F32, name=f"t{ci}", tag=f"t{ci}")
                for _ in range(2):
                    nc.vector.tensor_scalar_max(y, y, 1e-12)
                    nc.vector.reciprocal(r, y)
                    nc.vector.tensor_mul(t, sq, r)
                    nc.vector.tensor_add(t, t, y)
                    nc.vector.tensor_scalar_mul(y, t, 0.5)
                nc.vector.tensor_copy(D_sd[ci], y)
                ge = rot.tile([pn, MY], F32, name=f"ge{ci}", tag=f"ge{ci}")
                nc.vector.tensor_scalar(ge, y, CUT, None, op0=OP.is_ge)
                eyec = eye0 if ci == 0 else eye1
                nc.vector.tensor_tensor(om[ci], ge, eyec, op=OP.max)
                nc.scalar.activation(mask_s[ci], om[ci], AF.Identity,
                                     bias=1.0, scale=-1.0)

            D_flat = stat.tile([1, PAIRS], F32, name="D_flat", tag="D_flat")
            nc.sync.dma_start(
                D_flat[0:1, 0:128 * MY],
                bass.AP(tensor=D_sd[0].tensor, offset=D_sd[0].offset,
                        ap=[D_sd[0].ap[0], [1, MY]]))
            nc.sync.dma_start(
                D_flat[0:1, 128 * MY:PAIRS],
                bass.AP(tensor=D_sd[1].tensor, offset=D_sd[1].offset,
                        ap=[D_sd[1].ap[0], [1, MY]]))
            nc.sync.dma_start(
                slab[112:113, 0:128 * MY],
                bass.AP(tensor=om[0].tensor, offset=om[0].offset,
                        ap=[om[0].ap[0], [1, MY]]))
            nc.sync.dma_start(
                slab[112:113, 128 * MY:PAIRS],
                bass.AP(tensor=om[1].tensor, offset=om[1].offset,
                        ap=[om[1].ap[0], [1, MY]]))

            cnt_ps = psB.tile([1, MY], F32, tag="small", name="cnt_ps")
            nc.tensor.matmul(cnt_ps, ones_col[0:128, 0:1], mask_s[0],
                             start=True, stop=False)
            nc.tensor.matmul(cnt_ps, ones_col[0:64, 0:1], mask_s[1],
                             start=False, stop=True)
            cnt = stat.tile([1, MY], F32, name="cnt", tag="cnt")
            nc.vector.tensor_copy(cnt, cnt_ps)

            XsT0 = stat.tile([128, 3], F32, name="XsT0", tag="XsT0")
            XsT1 = stat.tile([64, 3], F32, name="XsT1", tag="XsT1")

            def rebuild_xsT(suffix):
                tp0 = psB.tile([128, 3], F32, tag="small", name=f"tp0_{suffix}")
                nc.tensor.matmul(tp0, Xf[:, 0:128], ident[0:3, 0:3],
                                 start=True, stop=True, is_transpose=True)
                nc.vector.tensor_copy(XsT0, tp0)
                tp1 = psB.tile([64, 3], F32, tag="small", name=f"tp1_{suffix}")
                nc.tensor.matmul(tp1, Xf[:, 128:192], ident[0:3, 0:3],
                                 start=True, stop=True, is_transpose=True)
                nc.vector.tensor_copy(XsT1, tp1)

            rebuild_xsT("init")

            w_sbuf = stat.tile([1, PAIRS], F32, name="w_sbuf", tag="w_sbuf")
            Wm0 = stat.tile([128, MY], F32, name="Wm0", tag="Wm0")
            Wm1 = stat.tile([64, MY], F32, name="Wm1", tag="Wm1")
            G = [stat.tile([ND, MY], F32, name="G0", tag="G0"),
                 stat.tile([ND, MY], F32, name="G1", tag="G1")]

            CCIN = ND * MY + 3 * MY
            n_cc = max(l_eff - 1, 0)
            cc_ins = [dram.tile([CCIN], F32, name=f"ccin{l}", tag=f"ccin{l}")
                      for l in range(n_cc)]
            cc_outs = [dram.tile([4 * CCIN], F32, name=f"ccout{l}", tag=f"ccout{l}")
                       for l in range(n_cc)]

            hT_cur, hT_nxt = hT_myA, hT_myB
            Xf_cur, Xf_nxt = Xf_myA, Xf_myB
            st_col = hT.ap[1][0]

            for l in range(l_eff):
                last = (l == l_eff - 1)
                skip_n = last and (l_eff == L) and not noskip

                # ---- b^T into lhsT rows 64..111 (psum at base partition 64) ----
                paths = [("c", cw1dT, constc, lhsT_c)]
                if not skip_n:
                    paths.append(("n", nw1dT, constn, lhsT_n))
                for pname, wdT, cst, lhsT_tiles in paths:
                    for ch in range(2):
                        sl = slice(l * NH + ch * ND, l * NH + (ch + 1) * ND)
                        bt_ps = psB.tile([MY, ND], F32, tag="small",
                                         name=f"btps_{pname}{ch}_{l}")
                        nc.tensor.matmul(bt_ps, hT_cur, wdT[:, sl],
                                         start=True, stop=False)
                        nc.tensor.matmul(bt_ps, ones_row[0:1, 0:MY],
                                         cst[0:1, sl], start=False, stop=True)
                        bt_sb = rot.tile([MY, ND], F32, tag="btsb",
                                         name=f"btsb_{pname}{ch}_{l}")
                        nc.vector.tensor_copy(bt_sb, bt_ps)
                        nc.sync.dma_start(lhsT_tiles[(l, ch)][64:112, :], bt_sb)

                # ---- e1s ----
                for (s0, ns) in _BLOCKS:
                    w0, wn = s0 * MY, ns * MY
                    e1_ps = psB.tile([NE, 480], F32, tag="small",
                                     name=f"e1ps_{l}_{s0}")
                    nc.tensor.matmul(
                        e1_ps[:, 0:wn], ew1[0:1, l * NE:(l + 1) * NE],
                        D_flat[0:1, w0:w0 + wn], start=True, stop=True)
                    nc.scalar.activation(
                        slab[0:NE, w0:w0 + wn], e1_ps[:, 0:wn], AF.Silu,
                        bias=eb1[:, l:l + 1])

                if not skip_n:
                    nc.vector.memset(G[0], 0.0)
                    nc.vector.memset(G[1], 0.0)

                # ---- pair supertiles ----
                for sti, blocks in enumerate(_SUPER):
                    nblk = len(blocks)
                    width = nblk * 512

                    def pre_mms(lhsT_tiles, w1sT, ch, name):
                        pre = psA.tile([128, width], F32, tag="big", name=name)
                        for bi, (s0, ns) in enumerate(blocks):
                            wn = ns * MY
                            out_ap = pre[:, bi * 512: bi * 512 + wn]
                            nc.tensor.matmul(
                                out_ap, lhsT_tiles[(l, ch)],
                                slab[0:113, s0 * MY: s0 * MY + wn],
                                start=True, stop=False)
                            rhs_rep = bass.AP(
                                tensor=hT.tensor,
                                offset=hT.offset + s0 * st_col,
                                ap=[hT.ap[0], [st_col, ns], [0, MY]])
                            nc.tensor.matmul(
                                out_ap,
                                w1sT[:, l * NH + ch * ND: l * NH + (ch + 1) * ND],
                                rhs_rep, start=False, stop=True)
                        return pre

                    if not skip_n:
                        for ch in range(2):
                            pre = pre_mms(lhsT_n, nw1sT, ch, f"pre_n{ch}_{l}_{sti}")
                            nc.scalar.activation(pre, pre, AF.Silu)
                            part = rot.tile([ND, MY], F32, tag="gpart",
                                            name=f"gp_{ch}_{l}_{sti}")
                            if blocks[-1][1] == blocks[0][1]:
                                red_ap = bass.AP(
                                    tensor=pre.tensor, offset=pre.offset,
                                    ap=[pre.ap[0], [1, MY], [512, nblk],
                                        [MY, blocks[0][1]]])
                                nc.vector.tensor_reduce(
                                    part, red_ap, axis=mybir.AxisListType.XY,
                                    op=OP.add)
                                nc.vector.tensor_add(G[ch], G[ch], part)
                            else:
                                red_ap = bass.AP(
                                    tensor=pre.tensor, offset=pre.offset,
                                    ap=[pre.ap[0], [1, MY], [512, nblk - 1],
                                        [MY, blocks[0][1]]])
                                nc.vector.tensor_reduce(
                                    part, red_ap, axis=mybir.AxisListType.XY,
                                    op=OP.add)
                                nc.vector.tensor_add(G[ch], G[ch], part)
                                red2 = bass.AP(
                                    tensor=pre.tensor,
                                    offset=pre.offset + (nblk - 1) * 512,
                                    ap=[pre.ap[0], [1, MY], [MY, blocks[-1][1]]])
                                part2 = rot.tile([ND, MY], F32, tag="gpart",
                                                 name=f"gp2_{ch}_{l}_{sti}")
                                nc.vector.tensor_reduce(
                                    part2, red2, axis=mybir.AxisListType.X,
                                    op=OP.add)
                                nc.vector.tensor_add(G[ch], G[ch], part2)

                    sp_cs = []
                    for ch in range(2):
                        pre = pre_mms(lhsT_c, cw1sT, ch, f"pre_c{ch}_{l}_{sti}")
                        sp_c = rot.tile([128, 1536], F32, tag="spc",
                                        name=f"spc_{ch}_{l}_{sti}")
                        nc.scalar.activation(sp_c[:, 0:width], pre, AF.Silu)
                        sp_cs.append(sp_c)
                    for bi, (s0, ns) in enumerate(blocks):
                        wn = ns * MY
                        wp = psB.tile([1, 480], F32, tag="small",
                                      name=f"wp_{l}_{sti}_{bi}")
                        nc.tensor.matmul(
                            wp[:, 0:wn], cw2[:, (l * 2):(l * 2) + 1],
                            sp_cs[0][:, bi * 512: bi * 512 + wn],
                            start=True, stop=False)
                        nc.tensor.matmul(
                            wp[:, 0:wn], cw2[:, (l * 2 + 1):(l * 2 + 1) + 1],
                            sp_cs[1][:, bi * 512: bi * 512 + wn],
                            start=False, stop=True)
                        nc.vector.tensor_copy(
                            w_sbuf[0:1, s0 * MY: s0 * MY + wn], wp[:, 0:wn])

                # ---- x aggregation ----
                nc.sync.dma_start(
                    Wm0,
                    bass.AP(tensor=w_sbuf.tensor, offset=w_sbuf.offset,
                            ap=[w_sbuf.ap[0], [MY, 128], [1, MY]]))
                nc.sync.dma_start(
                    Wm1,
                    bass.AP(tensor=w_sbuf.tensor,
                            offset=w_sbuf.offset + 128 * MY,
                            ap=[w_sbuf.ap[0], [MY, 64], [1, MY]]))
                xa_ps = psB.tile([3, MY], F32, tag="small", name=f"xa_{l}")
                nc.tensor.matmul(xa_ps, XsT0, Wm0, start=True, stop=False)
                nc.tensor.matmul(xa_ps, XsT1, Wm1, start=False, stop=True)
                ss_ps = psB.tile([3, MY], F32, tag="small", name=f"ss_{l}")
                nc.tensor.matmul(ss_ps, ones_col[0:128, :], Wm0,
                                 start=True, stop=False)
                nc.tensor.matmul(ss_ps, ones_col[0:64, :], Wm1,
                                 start=False, stop=True)
                t1 = rot.tile([3, MY], F32, name=f"t1_{l}", tag="t1")
                nc.vector.tensor_mul(t1, Xf_cur, ss_ps)
                nc.vector.tensor_sub(t1, xa_ps, t1)
                nc.vector.tensor_add(Xf_nxt, Xf_cur, t1)

                # ---- h aggregation ----
                if not skip_n:
                    ha_ps = psB.tile([ND, MY], F32, tag="small", name=f"ha_{l}")
                    nc.tensor.matmul(
                        ha_ps, nw2T[:, (l * 2) * ND:(l * 2 + 1) * ND], G[0],
                        start=True, stop=False)
                    nc.tensor.matmul(
                        ha_ps, nw2T[:, (l * 2 + 1) * ND:(l * 2 + 2) * ND], G[1],
                        start=False, stop=False)
                    nc.tensor.matmul(
                        ha_ps, nb2[0:1, l * ND:(l + 1) * ND], cnt,
                        start=False, stop=True)
                    nc.vector.tensor_add(hT_nxt, hT_cur, ha_ps)

                # ---- inter-layer exchange ----
                if not last:
                    cc_in, cc_out = cc_ins[l], cc_outs[l]
                    nc.sync.dma_start(
                        cc_in[0:ND * MY].rearrange("(p f) -> p f", p=ND), hT_nxt)
                    nc.sync.dma_start(
                        cc_in[ND * MY:CCIN].rearrange("(p f) -> p f", p=3), Xf_nxt)
                    nc.gpsimd.collective_compute(
                        "AllGather", OP.bypass,
                        replica_groups=[[0, 1, 2, 3], [4, 5, 6, 7]],
                        ins=[cc_in[:]], outs=[cc_out[:]])
                    for r in range(4):
                        nc.sync.dma_start(
                            hT[:, r * MY:(r + 1) * MY],
                            cc_out[r * CCIN: r * CCIN + ND * MY]
                            .rearrange("(p f) -> p f", p=ND))
                        nc.sync.dma_start(
                            Xf[:, r * MY:(r + 1) * MY],
                            cc_out[r * CCIN + ND * MY: (r + 1) * CCIN]
                            .rearrange("(p f) -> p f", p=3))
                    rebuild_xsT(str(l))

                hT_cur, hT_nxt = hT_nxt, hT_cur
                Xf_cur, Xf_nxt = Xf_nxt, Xf_cur

            nc.sync.dma_start(d_out[:], Xf_cur)
            if debug_h:
                nc.sync.dma_start(d_hout[:], hT_cur)

    nc.compile()
    return nc


def _marshal(inputs):
    g = {k: np.asarray(v, np.float32) for k, v in inputs.items()}
    h = g["h"]
    init_w = g["init_w"]
    init_b = g["init_b"]
    ew1 = g["edge_w1"]
    eb1 = g["edge_b1"]
    ew2 = g["edge_w2"]
    eb2 = g["edge_b2"]
    nw1 = g["node_w1"]
    nb1 = g["node_b1"]
    nw2 = g["node_w2"]
    nb2 = g["node_b2"]
    cw1 = g["coord_w1"]
    cb1 = g["coord_b1"]
    cw2 = g["coord_w2"]

    ew1_pack = np.zeros((1, L * NE), np.float32)
    eb1_pack = np.zeros((NE, L), np.float32)
    wnext_pack = np.zeros((65, L * NH), np.float32)
    wcext_pack = np.zeros((65, L * NH), np.float32)
    nw1sT_pack = np.zeros((ND, L * NH), np.float32)
    cw1sT_pack = np.zeros((ND, L * NH), np.float32)
    nw1dT_pack = np.zeros((ND, L * NH), np.float32)
    cw1dT_pack = np.zeros((ND, L * NH), np.float32)
    constn_pack = np.zeros((1, L * NH), np.float32)
    constc_pack = np.zeros((1, L * NH), np.float32)
    nw2T_pack = np.zeros((ND, L * 2 * ND), np.float32)
    nb2_pack = np.zeros((1, L * ND), np.float32)
    cw2_pack = np.zeros((ND, L * 2), np.float32)

    for l in range(L):
        nw1_s, nw1_d, nw1_e = nw1[l][:, :ND], nw1[l][:, ND:2 * ND], nw1[l][:, 2 * ND:]
        cw1_s, cw1_d, cw1_e = cw1[l][:, :ND], cw1[l][:, ND:2 * ND], cw1[l][:, 2 * ND:]
        W_ne = nw1_e @ ew2[l]
        W_ce = cw1_e @ ew2[l]
        sl = slice(l * NH, (l + 1) * NH)
        ew1_pack[0, l * NE:(l + 1) * NE] = ew1[l][:, 0]
        eb1_pack[:, l] = eb1[l]
        wnext_pack[0:64, sl] = W_ne.T
        wnext_pack[64, sl] = -BIG
        wcext_pack[0:64, sl] = W_ce.T
        wcext_pack[64, sl] = -BIG
        nw1sT_pack[:, sl] = nw1_s.T
        cw1sT_pack[:, sl] = cw1_s.T
        nw1dT_pack[:, sl] = nw1_d.T
        cw1dT_pack[:, sl] = cw1_d.T
        constn_pack[0, sl] = nb1[l] + nw1_e @ eb2[l]
        constc_pack[0, sl] = cb1[l] + cw1_e @ eb2[l]
        nw2T_l = nw2[l].T
        nw2T_pack[:, (l * 2) * ND:(l * 2 + 1) * ND] = nw2T_l[0:ND, :]
        nw2T_pack[:, (l * 2 + 1) * ND:(l * 2 + 2) * ND] = nw2T_l[ND:, :]
        nb2_pack[0, l * ND:(l + 1) * ND] = nb2[l]
        cw2_pack[:, l * 2] = cw2[l][0, 0:ND]
        cw2_pack[:, l * 2 + 1] = cw2[l][0, ND:]

    idx = np.arange(PAIRS)
    onehot = (idx[None, :] % MY == np.arange(MY)[:, None]).astype(np.float32)
    ident128 = np.eye(ND, dtype=np.float32)
    eye_full = np.eye(N, dtype=np.float32)

    shared = dict(
        onehot_slab=onehot, ident128=ident128,
        init_wT=np.ascontiguousarray(init_w.T),
        init_b_row=np.ascontiguousarray(init_b[None, :]),
        ew1_pack=ew1_pack, eb1_pack=eb1_pack,
        wnext_pack=wnext_pack, wcext_pack=wcext_pack,
        nw1sT_pack=nw1sT_pack, cw1sT_pack=cw1sT_pack,
        nw1dT_pack=nw1dT_pack, cw1dT_pack=cw1dT_pack,
        constn_pack=constn_pack, constc_pack=constc_pack,
        nw2T_pack=nw2T_pack, nb2_pack=nb2_pack, cw2_pack=cw2_pack,
    )

    in_maps = []
    for c in range(N_CORES):
        gi, ii = c // 4, c % 4
        sl = slice(ii * MY, (ii + 1) * MY)
        m = dict(shared)
        m["hT"] = np.ascontiguousarray(h[gi].T)
        m["hT_my"] = np.ascontiguousarray(h[gi][sl].T)
        m["eye_my"] = np.ascontiguousarray(eye_full[:, sl])
        in_maps.append(m)
    return in_maps


_CACHE = {}


def _get_nc(l_eff=L, debug_h=False, noskip=False):
    key = (l_eff, debug_h, noskip)
    if key not in _CACHE:
        _CACHE[key] = _build(l_eff, debug_h, noskip)
    return _CACHE[key]


def _run(inputs, l_eff=L, debug_h=False, trace=False, noskip=False):
    from concourse import bass_utils

    nc = _get_nc(l_eff, debug_h, noskip)
    in_maps = _marshal(inputs)
    res = bass_utils.run_bass_kernel_spmd(
        nc, in_maps, core_ids=list(range(N_CORES)), trace=trace)
    out = np.zeros((B, N, 3), np.float32)
    for c in range(N_CORES):
        gi, ii = c // 4, c % 4
        out[gi, ii * MY:(ii + 1) * MY, :] = res.results[c]["x_out"].T
    if debug_h:
        hout = np.zeros((B, N, ND), np.float32)
        for c in range(N_CORES):
            gi, ii = c // 4, c % 4
            hout[gi, ii * MY:(ii + 1) * MY, :] = res.results[c]["h_out"].T
        return out, hout, res
    return out, res


def kernel(**inputs):
    out, _ = _run(inputs)
    return out


# revision 17
# speedup vs baseline: 153.6156x; 153.6156x over previous
"""Trainium2 Bass kernel for nn_EquivariantDecoder (EGNN, B=2, N=192, 4 layers).

Sharding: 8 cores = 2 graphs x 4 dst-chunks of 48. Each core computes all
(src, dst) pairs for its 48 dsts; h/x are re-assembled between layers with an
AllGather over the 4-core group of each graph.

Algebraic restructuring (vs the dense reference):
- edge-MLP second layer fused into the pair-MLP first layers:
    W_ne = nw1_e @ ew2 (host-side), const_n = nb1 + nw1_e @ eb2
- masked pairs: a dedicated contraction channel carries -BIG into the
  pre-activation, so silu(pre) ~= 0 exactly for masked pairs (no per-pair
  mask multiply anywhere downstream)
- h[dst]-projection bias b[h,d] folded into the SAME first matmul via 48
  extra contraction channels (lhsT rows 64..111 = b^T, rhs rows = onehot(d))
- aggregation swapped before the second matmul:
    h_agg = nw2 @ (sum_s silu(pre_n)) + nb2 * cnt(d)
- coordinate update factored:  x_agg[:,d] = X @ w'[:,d] - x[:,d] * S(d)
  with w'[s,d] = cw2 @ silu(pre_c) (auto-masked by -BIG), S = sum_s w'.

Pair order is s-major: p = s*48 + d_local. PSUM blocks hold 10 srcs x 48 dsts
(480 cols) per 512-col bank.

Slab layout [113, 9216] (shared rhs for the first matmuls, K=113):
  rows 0..63   e1s = silu(D*ew1 + eb1)  (rewritten per layer)
  rows 64..111 onehot(d_local) pattern  (static)
  row  112     1 - mask                 (static; lhsT row = -BIG)
"""

import numpy as np

# ---- problem constants (hardcoded; kernel.py must be self-contained) ----
B = 2
N = 192
ND = 128          # node dim
NH = 256          # hidden
NE = 64           # edge dim
L = 4
CUT = 2.0
MY = 48           # dsts per core
BIG = 1e30
N_CORES = 8
PAIRS = N * MY    # 9216

# s-blocks: 10 srcs x 48 dsts = 480 cols per PSUM bank
_BLOCKS = [(s0, min(10, N - s0)) for s0 in range(0, N, 10)]  # 19x10 + 1x2
# supertiles: up to 3 blocks each
_SUPER = [_BLOCKS[i:i + 2] for i in range(0, len(_BLOCKS), 2)]


def _build(l_eff=L, debug_h=False, noskip=False, no_cc=False):
    import concourse.bass as bass
    import concourse.bacc as bacc
    import concourse.tile as tile
    import concourse.mybir as mybir

    F32 = mybir.dt.float32
    F32R = mybir.dt.float32r
    AF = mybir.ActivationFunctionType
    OP = mybir.AluOpType

    nc = bacc.Bacc(
        "TRN2",
        target_bir_lowering=False,
        debug=False,
        enable_asserts=False,
        num_devices=N_CORES,
    )

    def din(name, shape):
        return nc.dram_tensor(name, shape, F32, kind="ExternalInput")

    d_hT = din("hT", [ND, N])
    d_hT_my = din("hT_my", [ND, MY])
    d_eye = din("eye_my", [N, MY])
    d_onehot = din("onehot_slab", [MY, PAIRS])
    d_ident = din("ident128", [ND, ND])
    d_initwT = din("init_wT", [ND, 3])
    d_initb = din("init_b_row", [1, 3])
    d_ew1 = din("ew1_pack", [1, L * NE])
    d_eb1 = din("eb1_pack", [NE, L])
    d_wnext = din("wnext_pack", [113, L * NH])
    d_wcext = din("wcext_pack", [113, L * NH])
    d_nw1sT = din("nw1sT_pack", [ND, L * NH])
    d_cw1sT = din("cw1sT_pack", [ND, L * NH])
    d_nw1dT = din("nw1dT_pack", [ND, L * NH])
    d_cw1dT = din("cw1dT_pack", [ND, L * NH])
    d_constn = din("constn_pack", [1, L * NH])
    d_constc = din("constc_pack", [1, L * NH])
    d_nw2T = din("nw2T_pack", [ND, L * 2 * ND])
    d_nb2 = din("nb2_pack", [1, L * ND])
    d_cw2 = din("cw2_pack", [ND, L * 2])

    d_out = nc.dram_tensor("x_out", [3, MY], F32, kind="ExternalOutput")
    d_hout = None
    if debug_h:
        d_hout = nc.dram_tensor("h_out", [ND, MY], F32, kind="ExternalOutput")

    with tile.TileContext(nc) as tc:
        with (
            tc.tile_pool(name="stat", bufs=1) as stat,
            tc.tile_pool(name="rot", bufs=3) as rot,
            tc.tile_pool(name="psA", bufs=3, space="PSUM") as psA,
            tc.tile_pool(name="psB", bufs=2, space="PSUM") as psB,
            tc.tile_pool(name="dram", bufs=1, space="DRAM") as dram,
        ):
            # ---------------- static SBUF loads ----------------
            def sload(d, shape, name):
                t = stat.tile(shape, F32, name=name, tag=name)
                nc.sync.dma_start(t, d[:])
                return t

            hT = sload(d_hT, [ND, N], "hT")
            hT_r = stat.tile([ND, N], mybir.dt.float32r, name="hT_r", tag="hT_r")
            nc.vector.tensor_copy(hT_r, hT)
            hT_myA = sload(d_hT_my, [ND, MY], "hT_myA")
            hT_myB = stat.tile([ND, MY], F32, name="hT_myB", tag="hT_myB")
            eye0 = stat.tile([128, MY], F32, name="eye0", tag="eye0")
            eye1 = stat.tile([64, MY], F32, name="eye1", tag="eye1")
            nc.sync.dma_start(eye0, d_eye[0:128, :])
            nc.sync.dma_start(eye1, d_eye[128:192, :])
            ident = sload(d_ident, [ND, ND], "ident")
            initwT = sload(d_initwT, [ND, 3], "initwT")
            initb = sload(d_initb, [1, 3], "initb")
            ew1 = sload(d_ew1, [1, L * NE], "ew1")
            eb1 = sload(d_eb1, [NE, L], "eb1")
            nw1sT_r = stat.tile([ND, L * NH], mybir.dt.float32r,
                                name="nw1sT_r", tag="nw1sT_r")
            t_st = rot.tile([ND, L * NH], F32, tag="packst", name="nw1sT_st")
            nc.sync.dma_start(t_st, d_nw1sT[:])
            nc.vector.tensor_copy(nw1sT_r, t_st)
            cw1sT_r = stat.tile([ND, L * NH], mybir.dt.float32r,
                                name="cw1sT_r", tag="cw1sT_r")
            t_st2 = rot.tile([ND, L * NH], F32, tag="packst", name="cw1sT_st")
            nc.sync.dma_start(t_st2, d_cw1sT[:])
            nc.vector.tensor_copy(cw1sT_r, t_st2)
            nw1dT = sload(d_nw1dT, [ND, L * NH], "nw1dT")
            cw1dT = sload(d_cw1dT, [ND, L * NH], "cw1dT")
            constn = sload(d_constn, [1, L * NH], "constn")
            constc = sload(d_constc, [1, L * NH], "constc")
            nw2T = sload(d_nw2T, [ND, L * 2 * ND], "nw2T")
            nb2 = sload(d_nb2, [1, L * ND], "nb2")
            cw2 = sload(d_cw2, [ND, L * 2], "cw2")

            slab = stat.tile([113, PAIRS], F32R, name="slab", tag="slab")
            for w0 in range(0, PAIRS, 1024):
                stg = rot.tile([32, 1024], F32, tag="stgsm", name=f"stg_{w0}")
                nc.sync.dma_start(stg, d_onehot[0:32, w0:w0 + 1024])
                nc.vector.tensor_copy(slab[64:96, w0:w0 + 1024], stg)

            # pack tiles then cast into F32R lhsT tiles [113, 128]
            wnext = rot.tile([113, L * NH], F32, tag="packst", name="wnext_st")
            nc.sync.dma_start(wnext, d_wnext[:])
            wcext = rot.tile([113, L * NH], F32, tag="packst", name="wcext_st")
            nc.sync.dma_start(wcext, d_wcext[:])
            lhsT_n = {}
            lhsT_c = {}
            for l in range(l_eff):
                for ch in range(2):
                    sl = slice(l * NH + ch * ND, l * NH + (ch + 1) * ND)
                    tn = stat.tile([113, ND], F32R, name=f"lhsTn_{l}_{ch}",
                                   tag=f"lhsTn_{l}_{ch}")
                    nc.vector.tensor_copy(tn[0:64, :], wnext[0:64, sl])
                    nc.vector.tensor_copy(tn[96:113, :], wnext[96:113, sl])
                    lhsT_n[(l, ch)] = tn
                    tcn = stat.tile([113, ND], F32R, name=f"lhsTc_{l}_{ch}",
                                    tag=f"lhsTc_{l}_{ch}")
                    nc.vector.tensor_copy(tcn[0:64, :], wcext[0:64, sl])
                    nc.vector.tensor_copy(tcn[96:113, :], wcext[96:113, sl])
                    lhsT_c[(l, ch)] = tcn

            ones_row = stat.tile([1, N], F32, name="ones_row", tag="ones_row")
            nc.vector.memset(ones_row, 1.0)
            ones_col = stat.tile([128, 3], F32, name="ones_col", tag="ones_col")
            nc.vector.memset(ones_col, 1.0)

            # ---------------- init: coords, D, mask ----------------
            xf_ps = psB.tile([3, N], F32, tag="small", name="xf_ps")
            nc.tensor.matmul(xf_ps, initwT, hT, start=True, stop=False)
            nc.tensor.matmul(xf_ps, initb, ones_row, start=False, stop=True)
            Xf = stat.tile([3, N], F32, name="Xf", tag="Xf")
            nc.vector.tensor_copy(Xf, xf_ps)

            xfm_ps = psB.tile([3, MY], F32, tag="small", name="xfm_ps")
            nc.tensor.matmul(xfm_ps, initwT, hT_myA, start=True, stop=False)
            nc.tensor.matmul(xfm_ps, initb, ones_row[0:1, 0:MY],
                             start=False, stop=True)
            Xf_myA = stat.tile([3, MY], F32, name="Xf_myA", tag="Xf_myA")
            Xf_myB = stat.tile([3, MY], F32, name="Xf_myB", tag="Xf_myB")
            nc.vector.tensor_copy(Xf_myA, xfm_ps)

            # n2 = sum_i x_i^2 as [1, N] via K=3 matmul with ones
            sqX = stat.tile([3, N], F32, name="sqX", tag="sqX")
            nc.vector.tensor_mul(sqX, Xf, Xf)
            n2_ps = psB.tile([1, N], F32, tag="small", name="n2_ps")
            nc.tensor.matmul(n2_ps, ones_col[0:3, 0:1], sqX, start=True, stop=True)
            n2 = stat.tile([1, N], F32, name="n2", tag="n2")
            nc.vector.tensor_copy(n2, n2_ps)
            sqXm = stat.tile([3, MY], F32, name="sqXm", tag="sqXm")
            nc.vector.tensor_mul(sqXm, Xf_myA, Xf_myA)
            n2m_ps = psB.tile([1, MY], F32, tag="small", name="n2m_ps")
            nc.tensor.matmul(n2m_ps, ones_col[0:3, 0:1], sqXm, start=True, stop=True)
            n2m = stat.tile([1, MY], F32, name="n2m", tag="n2m")
            nc.vector.tensor_copy(n2m, n2m_ps)

            # P [5, N]: rows 0-2 = -2x, row 3 = ones (DMA), row 4 = n2 (DMA)
            P = stat.tile([5, N], F32, name="P", tag="P")
            nc.vector.tensor_scalar_mul(P[0:3, :], Xf, -2.0)
            nc.sync.dma_start(P[3:4, :], ones_row)
            nc.sync.dma_start(P[4:5, :], n2)
            # Q [5, MY]: rows 0-2 = x_my (DMA), row 3 = n2m (DMA), row 4 = ones
            Q = stat.tile([5, MY], F32, name="Q", tag="Q")
            nc.vector.tensor_copy(Q[0:3, :], Xf_myA)
            nc.sync.dma_start(Q[3:4, :], n2m)
            nc.sync.dma_start(Q[4:5, :], ones_row[0:1, 0:MY])

            D_sd = [stat.tile([128, MY], F32, name="D0", tag="D0"),
                    stat.tile([64, MY], F32, name="D1", tag="D1")]
            om = [stat.tile([128, MY], F32, name="om0", tag="om0"),
                  stat.tile([64, MY], F32, name="om1", tag="om1")]
            mask_s = [stat.tile([128, MY], F32, name="ms0", tag="ms0"),
                      stat.tile([64, MY], F32, name="ms1", tag="ms1")]
            for ci, (p0, pn) in enumerate([(0, 128), (128, 64)]):
                sq_ps = psB.tile([pn, MY], F32, tag="small", name=f"sqps{ci}")
                nc.tensor.matmul(sq_ps, P[:, p0:p0 + pn], Q, start=True, stop=True)
                sq = rot.tile([pn, MY], F32, name=f"sq{ci}", tag=f"sq{ci}")
                nc.vector.tensor_scalar_max(sq, sq_ps, 0.0)
                y = rot.tile([pn, MY], F32, name=f"y{ci}", tag=f"y{ci}")
                nc.scalar.activation(y, sq, AF.Sqrt)
                r = rot.tile([pn, MY], F32, name=f"r{ci}", tag=f"r{ci}")
                t = rot.tile([pn, MY], F32, name=f"t{ci}", tag=f"t{ci}")
                for _ in range(2):
                    nc.vector.tensor_scalar_max(y, y, 1e-12)
                    nc.vector.reciprocal(r, y)
                    nc.vector.tensor_mul(t, sq, r)
                    nc.vector.tensor_add(t, t, y)
                    nc.vector.tensor_scalar_mul(y, t, 0.5)
                nc.vector.tensor_copy(D_sd[ci], y)
                ge = rot.tile([pn, MY], F32, name=f"ge{ci}", tag=f"ge{ci}")
                nc.vector.tensor_scalar(ge, y, CUT, None, op0=OP.is_ge)
                eyec = eye0 if ci == 0 else eye1
                nc.vector.tensor_tensor(om[ci], ge, eyec, op=OP.max)
                nc.scalar.activation(mask_s[ci], om[ci], AF.Identity,
                                     bias=1.0, scale=-1.0)

            D_flat_r = stat.tile([1, PAIRS], mybir.dt.float32r,
                                 name="D_flat_r", tag="D_flat_r")
            for ci, base, cnt in [(0, 0, 128), (1, 128, 64)]:
                for s0 in range(0, cnt, 16):
                    p0 = (base + s0) * MY
                    stg2 = rot.tile([1, 768], F32, tag="stgsm2",
                                    name=f"stg2_{ci}_{s0}")
                    nc.sync.dma_start(stg2[0:1, :], D_sd[ci][s0:s0 + 16, :])
                    nc.vector.tensor_copy(D_flat_r[0:1, p0:p0 + 768], stg2[0:1, :])
                    s17 = rot.tile([113, 768], F32, tag="stg17",
                                   name=f"s17_{ci}_{s0}")
                    nc.sync.dma_start(s17[96:112, :], d_onehot[32:48, p0:p0 + 768])
                    nc.sync.dma_start(s17[112:113, :], om[ci][s0:s0 + 16, :])
                    nc.vector.tensor_copy(slab[96:113, p0:p0 + 768],
                                          s17[96:113, :])
            ew1_r = stat.tile([1, L * NE], mybir.dt.float32r,
                              name="ew1_r", tag="ew1_r")
            nc.vector.tensor_copy(ew1_r, ew1)
            cw2_r = stat.tile([ND, L * 2], mybir.dt.float32r,
                              name="cw2_r", tag="cw2_r")
            nc.vector.tensor_copy(cw2_r, cw2)

            cnt_ps = psB.tile([1, MY], F32, tag="small", name="cnt_ps")
            nc.tensor.matmul(cnt_ps, ones_col[0:128, 0:1], mask_s[0],
                             start=True, stop=False)
            nc.tensor.matmul(cnt_ps, ones_col[0:64, 0:1], mask_s[1],
                             start=False, stop=True)
            cnt = stat.tile([1, MY], F32, name="cnt", tag="cnt")
            nc.vector.tensor_copy(cnt, cnt_ps)

            XsT0 = stat.tile([128, 3], F32, name="XsT0", tag="XsT0")
            XsT1 = stat.tile([64, 3], F32, name="XsT1", tag="XsT1")

            def rebuild_xsT(suffix):
                tp0 = psB.tile([128, 3], F32, tag="small", name=f"tp0_{suffix}")
                nc.tensor.matmul(tp0, Xf[:, 0:128], ident[0:3, 0:3],
                                 start=True, stop=True, is_transpose=True)
                nc.vector.tensor_copy(XsT0, tp0)
                tp1 = psB.tile([64, 3], F32, tag="small", name=f"tp1_{suffix}")
                nc.tensor.matmul(tp1, Xf[:, 128:192], ident[0:3, 0:3],
                                 start=True, stop=True, is_transpose=True)
                nc.vector.tensor_copy(XsT1, tp1)

            rebuild_xsT("init")

            Wm0 = stat.tile([128, MY], F32, name="Wm0", tag="Wm0")
            Wm1 = stat.tile([64, MY], F32, name="Wm1", tag="Wm1")
            G = [stat.tile([ND, MY], F32, name="G0", tag="G0"),
                 stat.tile([ND, MY], F32, name="G1", tag="G1")]

            CCIN = ND * MY + 3 * MY
            n_cc = max(l_eff - 1, 0)
            cc_ins = [dram.tile([CCIN], F32, name=f"ccin{l}", tag=f"ccin{l}")
                      for l in range(n_cc)]
            cc_outs = [dram.tile([4 * CCIN], F32, name=f"ccout{l}", tag=f"ccout{l}")
                       for l in range(n_cc)]

            hT_cur, hT_nxt = hT_myA, hT_myB
            Xf_cur, Xf_nxt = Xf_myA, Xf_myB
            st_col = hT.ap[1][0]

            for l in range(l_eff):
                last = (l == l_eff - 1)
                skip_n = last and (l_eff == L) and not noskip

                # ---- b^T into lhsT rows 64..111 (psum at base partition 64) ----
                paths = [("c", cw1dT, constc, lhsT_c)]
                if not skip_n:
                    paths.append(("n", nw1dT, constn, lhsT_n))
                for pname, wdT, cst, lhsT_tiles in paths:
                    for ch in range(2):
                        sl = slice(l * NH + ch * ND, l * NH + (ch + 1) * ND)
                        bt_ps = psB.tile([MY, ND], F32, tag="small",
                                         name=f"btps_{pname}{ch}_{l}")
                        nc.tensor.matmul(bt_ps, hT_cur, wdT[:, sl],
                                         start=True, stop=False)
                        nc.tensor.matmul(bt_ps, ones_row[0:1, 0:MY],
                                         cst[0:1, sl], start=False, stop=True)
                        bt_sb = rot.tile([MY, ND], F32, tag="btsb",
                                         name=f"btsb_{pname}{ch}_{l}")
                        nc.vector.tensor_copy(bt_sb, bt_ps)
                        bt_st = rot.tile([112, ND], F32, tag="btst",
                                         name=f"btst_{pname}{ch}_{l}")
                        nc.sync.dma_start(bt_st[64:112, :], bt_sb)
                        nc.vector.tensor_copy(lhsT_tiles[(l, ch)][64:112, :],
                                              bt_st[64:112, :])

                # ---- e1s ----
                for (s0, ns) in _BLOCKS:
                    w0, wn = s0 * MY, ns * MY
                    e1_ps = psB.tile([NE, 480], F32, tag="small",
                                     name=f"e1ps_{l}_{s0}")
                    nc.tensor.matmul(
                        e1_ps[:, 0:wn], ew1_r[0:1, l * NE:(l + 1) * NE],
                        D_flat_r[0:1, w0:w0 + wn], start=True, stop=True)
                    nc.scalar.activation(
                        slab[0:NE, w0:w0 + wn], e1_ps[:, 0:wn], AF.Silu,
                        bias=eb1[:, l:l + 1])

                if not skip_n:
                    nc.vector.memset(G[0], 0.0)
                    nc.vector.memset(G[1], 0.0)

                # ---- pair supertiles ----
                for sti, blocks in enumerate(_SUPER):
                    nblk = len(blocks)
                    width = nblk * 512

                    def pre_mms(lhsT_tiles, w1sT, ch, name):
                        pre = psA.tile([128, width], F32, tag="big", name=name)
                        for bi, (s0, ns) in enumerate(blocks):
                            wn = ns * MY
                            nc.tensor.matmul(
                                pre[:, bi * 512: bi * 512 + wn],
                                lhsT_tiles[(l, ch)],
                                slab[0:113, s0 * MY: s0 * MY + wn],
                                start=True, stop=False)
                        for bi, (s0, ns) in enumerate(blocks):
                            wn = ns * MY
                            rhs_rep = bass.AP(
                                tensor=hT_r.tensor,
                                offset=hT_r.offset + s0 * st_col,
                                ap=[hT_r.ap[0], [st_col, ns], [0, MY]])
                            nc.tensor.matmul(
                                pre[:, bi * 512: bi * 512 + wn],
                                w1sT[:, l * NH + ch * ND: l * NH + (ch + 1) * ND],
                                rhs_rep, start=False, stop=True)
                        return pre

                    if not skip_n:
                        for ch in range(2):
                            pre = pre_mms(lhsT_n, nw1sT_r, ch, f"pre_n{ch}_{l}_{sti}")
                            nc.scalar.activation(pre, pre, AF.Silu)
                            part = rot.tile([ND, MY], F32, tag="gpart",
                                            name=f"gp_{ch}_{l}_{sti}")
                            if blocks[-1][1] == blocks[0][1]:
                                red_ap = bass.AP(
                                    tensor=pre.tensor, offset=pre.offset,
                                    ap=[pre.ap[0], [1, MY], [512, nblk],
                                        [MY, blocks[0][1]]])
                                nc.vector.tensor_reduce(
                                    part, red_ap, axis=mybir.AxisListType.XY,
                                    op=OP.add)
                                nc.vector.tensor_add(G[ch], G[ch], part)
                            else:
                                red_ap = bass.AP(
                                    tensor=pre.tensor, offset=pre.offset,
                                    ap=[pre.ap[0], [1, MY], [512, nblk - 1],
                                        [MY, blocks[0][1]]])
                                nc.vector.tensor_reduce(
                                    part, red_ap, axis=mybir.AxisListType.XY,
                                    op=OP.add)
                                nc.vector.tensor_add(G[ch], G[ch], part)
                                red2 = bass.AP(
                                    tensor=pre.tensor,
                                    offset=pre.offset + (nblk - 1) * 512,
                                    ap=[pre.ap[0], [1, MY], [MY, blocks[-1][1]]])
                                part2 = rot.tile([ND, MY], F32, tag="gpart",
                                                 name=f"gp2_{ch}_{l}_{sti}")
                                nc.vector.tensor_reduce(
                                    part2, red2, axis=mybir.AxisListType.X,
                                    op=OP.add)
                                nc.vector.tensor_add(G[ch], G[ch], part2)

                    sp_cs = []
                    for ch in range(2):
                        pre = pre_mms(lhsT_c, cw1sT_r, ch, f"pre_c{ch}_{l}_{sti}")
                        sp_c = rot.tile([128, 1024], mybir.dt.float32r, tag="spc", bufs=3,
                                        name=f"spc_{ch}_{l}_{sti}")
                        nc.scalar.activation(sp_c[:, 0:width], pre, AF.Silu)
                        sp_cs.append(sp_c)
                    wp = psA.tile([1, width], F32, tag="big",
                                  name=f"wp_{l}_{sti}")
                    for ch in range(2):
                        for bi, (s0, ns) in enumerate(blocks):
                            wn = ns * MY
                            nc.tensor.matmul(
                                wp[:, bi * 512: bi * 512 + wn],
                                cw2_r[:, (l * 2 + ch):(l * 2 + ch) + 1],
                                sp_cs[ch][:, bi * 512: bi * 512 + wn],
                                start=(ch == 0), stop=(ch == 1))
                    st_s0 = blocks[0][0]
                    st_sn = blocks[-1][0] + blocks[-1][1] - st_s0
                    w_buf = rot.tile([1, 1024], F32, tag="wbuf",
                                     name=f"wbuf_{l}_{sti}")
                    if blocks[-1][1] == blocks[0][1]:
                        src_ap = bass.AP(
                            tensor=wp.tensor, offset=wp.offset,
                            ap=[wp.ap[0], [512, len(blocks)],
                                [1, blocks[0][1] * MY]])
                        nc.vector.tensor_copy(w_buf[0:1, 0:st_sn * MY], src_ap)
                    else:
                        off = 0
                        for bi, (s0, ns) in enumerate(blocks):
                            wn = ns * MY
                            nc.vector.tensor_copy(
                                w_buf[0:1, off:off + wn],
                                wp[:, bi * 512: bi * 512 + wn])
                            off += wn
                    # scatter w_buf rows into Wm chunks (split at s=128)
                    for (pa, pb) in [(st_s0, min(st_s0 + st_sn, 128)),
                                     (max(st_s0, 128), st_s0 + st_sn)]:
                        if pb <= pa:
                            continue
                        dst = Wm0[pa:pb, :] if pb <= 128 else Wm1[pa - 128:pb - 128, :]
                        nc.sync.dma_start(
                            dst,
                            bass.AP(tensor=w_buf.tensor,
                                    offset=w_buf.offset + (pa - st_s0) * MY,
                                    ap=[w_buf.ap[0], [MY, pb - pa], [1, MY]]))

                # ---- x aggregation ----
                xa_ps = psB.tile([3, MY], F32, tag="small", name=f"xa_{l}")
                nc.tensor.matmul(xa_ps, XsT0, Wm0, start=True, stop=False)
                nc.tensor.matmul(xa_ps, XsT1, Wm1, start=False, stop=True)
                ss_ps = psB.tile([3, MY], F32, tag="small", name=f"ss_{l}")
                nc.tensor.matmul(ss_ps, ones_col[0:128, :], Wm0,
                                 start=True, stop=False)
                nc.tensor.matmul(ss_ps, ones_col[0:64, :], Wm1,
                                 start=False, stop=True)
                t1 = rot.tile([3, MY], F32, name=f"t1_{l}", tag="t1")
                nc.vector.tensor_mul(t1, Xf_cur, ss_ps)
                nc.vector.tensor_sub(t1, xa_ps, t1)
                nc.vector.tensor_add(Xf_nxt, Xf_cur, t1)

                # ---- h aggregation ----
                if not skip_n:
                    ha_ps = psB.tile([ND, MY], F32, tag="small", name=f"ha_{l}")
                    nc.tensor.matmul(
                        ha_ps, nw2T[:, (l * 2) * ND:(l * 2 + 1) * ND], G[0],
                        start=True, stop=False)
                    nc.tensor.matmul(
                        ha_ps, nw2T[:, (l * 2 + 1) * ND:(l * 2 + 2) * ND], G[1],
                        start=False, stop=False)
                    nc.tensor.matmul(
                        ha_ps, nb2[0:1, l * ND:(l + 1) * ND], cnt,
                        start=False, stop=True)
                    nc.vector.tensor_add(hT_nxt, hT_cur, ha_ps)

                # ---- inter-layer exchange ----
                if not last:
                    cc_in, cc_out = cc_ins[l], cc_outs[l]
                    nc.sync.dma_start(
                        cc_in[0:ND * MY].rearrange("(p f) -> p f", p=ND), hT_nxt)
                    nc.sync.dma_start(
                        cc_in[ND * MY:CCIN].rearrange("(p f) -> p f", p=3), Xf_nxt)
                    if no_cc:
                        for r in range(4):
                            nc.sync.dma_start(
                                cc_out[r * CCIN:(r + 1) * CCIN], cc_in[:])
                    else:
                        nc.gpsimd.collective_compute(
                            "AllGather", OP.bypass,
                            replica_groups=[[0, 1, 2, 3], [4, 5, 6, 7]],
                            ins=[cc_in[:]], outs=[cc_out[:]])
                    for r in range(4):
                        nc.sync.dma_start(
                            hT[:, r * MY:(r + 1) * MY],
                            cc_out[r * CCIN: r * CCIN + ND * MY]
                            .rearrange("(p f) -> p f", p=ND))
                        nc.sync.dma_start(
                            Xf[:, r * MY:(r + 1) * MY],
                            cc_out[r * CCIN + ND * MY: (r + 1) * CCIN]
                            .rearrange("(p f) -> p f", p=3))
                    nc.vector.tensor_copy(hT_r, hT)
                    rebuild_xsT(str(l))

                hT_cur, hT_nxt = hT_nxt, hT_cur
                Xf_cur, Xf_nxt = Xf_nxt, Xf_cur

            nc.sync.dma_start(d_out[:], Xf_cur)
            if debug_h:
                nc.sync.dma_start(d_hout[:], hT_cur)

    nc.compile()
    return nc


def _marshal(inputs):
    g = {k: np.asarray(v, np.float32) for k, v in inputs.items()}
    h = g["h"]
    init_w = g["init_w"]
    init_b = g["init_b"]
    ew1 = g["edge_w1"]
    eb1 = g["edge_b1"]
    ew2 = g["edge_w2"]
    eb2 = g["edge_b2"]
    nw1 = g["node_w1"]
    nb1 = g["node_b1"]
    nw2 = g["node_w2"]
    nb2 = g["node_b2"]
    cw1 = g["coord_w1"]
    cb1 = g["coord_b1"]
    cw2 = g["coord_w2"]

    ew1_pack = np.zeros((1, L * NE), np.float32)
    eb1_pack = np.zeros((NE, L), np.float32)
    wnext_pack = np.zeros((113, L * NH), np.float32)
    wcext_pack = np.zeros((113, L * NH), np.float32)
    nw1sT_pack = np.zeros((ND, L * NH), np.float32)
    cw1sT_pack = np.zeros((ND, L * NH), np.float32)
    nw1dT_pack = np.zeros((ND, L * NH), np.float32)
    cw1dT_pack = np.zeros((ND, L * NH), np.float32)
    constn_pack = np.zeros((1, L * NH), np.float32)
    constc_pack = np.zeros((1, L * NH), np.float32)
    nw2T_pack = np.zeros((ND, L * 2 * ND), np.float32)
    nb2_pack = np.zeros((1, L * ND), np.float32)
    cw2_pack = np.zeros((ND, L * 2), np.float32)

    for l in range(L):
        nw1_s, nw1_d, nw1_e = nw1[l][:, :ND], nw1[l][:, ND:2 * ND], nw1[l][:, 2 * ND:]
        cw1_s, cw1_d, cw1_e = cw1[l][:, :ND], cw1[l][:, ND:2 * ND], cw1[l][:, 2 * ND:]
        W_ne = nw1_e @ ew2[l]
        W_ce = cw1_e @ ew2[l]
        sl = slice(l * NH, (l + 1) * NH)
        ew1_pack[0, l * NE:(l + 1) * NE] = ew1[l][:, 0]
        eb1_pack[:, l] = eb1[l]
        wnext_pack[0:64, sl] = W_ne.T
        wnext_pack[112, sl] = -BIG
        wcext_pack[0:64, sl] = W_ce.T
        wcext_pack[112, sl] = -BIG
        nw1sT_pack[:, sl] = nw1_s.T
        cw1sT_pack[:, sl] = cw1_s.T
        nw1dT_pack[:, sl] = nw1_d.T
        cw1dT_pack[:, sl] = cw1_d.T
        constn_pack[0, sl] = nb1[l] + nw1_e @ eb2[l]
        constc_pack[0, sl] = cb1[l] + cw1_e @ eb2[l]
        nw2T_l = nw2[l].T
        nw2T_pack[:, (l * 2) * ND:(l * 2 + 1) * ND] = nw2T_l[0:ND, :]
        nw2T_pack[:, (l * 2 + 1) * ND:(l * 2 + 2) * ND] = nw2T_l[ND:, :]
        nb2_pack[0, l * ND:(l + 1) * ND] = nb2[l]
        cw2_pack[:, l * 2] = cw2[l][0, 0:ND]
        cw2_pack[:, l * 2 + 1] = cw2[l][0, ND:]

    idx = np.arange(PAIRS)
    onehot = (idx[None, :] % MY == np.arange(MY)[:, None]).astype(np.float32)
    ident128 = np.eye(ND, dtype=np.float32)
    eye_full = np.eye(N, dtype=np.float32)

    shared = dict(
        onehot_slab=onehot, ident128=ident128,
        init_wT=np.ascontiguousarray(init_w.T),
        init_b_row=np.ascontiguousarray(init_b[None, :]),
        ew1_pack=ew1_pack, eb1_pack=eb1_pack,
        wnext_pack=wnext_pack, wcext_pack=wcext_pack,
        nw1sT_pack=nw1sT_pack, cw1sT_pack=cw1sT_pack,
        nw1dT_pack=nw1dT_pack, cw1dT_pack=cw1dT_pack,
        constn_pack=constn_pack, constc_pack=constc_pack,
        nw2T_pack=nw2T_pack, nb2_pack=nb2_pack, cw2_pack=cw2_pack,
    )

    in_maps = []
    for c in range(N_CORES):
        gi, ii = c // 4, c % 4
        sl = slice(ii * MY, (ii + 1) * MY)
        m = dict(shared)
        m["hT"] = np.ascontiguousarray(h[gi].T)
        m["hT_my"] = np.ascontiguousarray(h[gi][sl].T)
        m["eye_my"] = np.ascontiguousarray(eye_full[:, sl])
        in_maps.append(m)
    return in_maps


_CACHE = {}


def _get_nc(l_eff=L, debug_h=False, noskip=False, no_cc=False):
    key = (l_eff, debug_h, noskip, no_cc)
    if key not in _CACHE:
        _CACHE[key] = _build(l_eff, debug_h, noskip, no_cc)
    return _CACHE[key]


def _run(inputs, l_eff=L, debug_h=False, trace=False, noskip=False):
    from concourse import bass_utils

    nc = _get_nc(l_eff, debug_h, noskip)
    in_maps = _marshal(inputs)
    res = bass_utils.run_bass_kernel_spmd(
        nc, in_maps, core_ids=list(range(N_CORES)), trace=trace)
    out = np.zeros((B, N, 3), np.float32)
    for c in range(N_CORES):
        gi, ii = c // 4, c % 4
        out[gi, ii * MY:(ii + 1) * MY, :] = res.results[c]["x_out"].T
    if debug_h:
        hout = np.zeros((B, N, ND), np.float32)
        for c in range(N_CORES):
            gi, ii = c // 4, c % 4
            hout[gi, ii * MY:(ii + 1) * MY, :] = res.results[c]["h_out"].T
        return out, hout, res
    return out, res


def kernel(**inputs):
    out, _ = _run(inputs)
    return out
